# revision 8
# baseline (speedup 1.0000x reference)
"""Trainium2 Bass kernel for CompactCenterLoss (B=4096, D=512, C=100, 8 cores).

Math notes (vs the reference):
  dist[i, j] = ||x_i - centers[t_j]|| depends on j only through the class
  t_j, so the first BxB table collapses to a [B, C] table D2:
      dist_ap[i] = D2[i, t_i]                      (all same-class j equal)
      dist_an[i] = min_{c present, c != t_i} D2[i, c]
  Only pdist(x, x) needs the full BxB compute. Its masked row sums are
  obtained per class via a matmul with the one-hot matrix O [B, C]:
      S^T[c, i] = sum_j O[j, c] * dist[j, i]
      pos_sum_i = S^T[t_i, i],  tot_sum_i = sum_c S^T[c, i]

Sharding: batch rows are split across 8 cores (512 rows each). Every core
computes dist^T tiles [128 j x 512 i_shard] against the full (replicated)
input, using a per-core ROTATION of the j axis so the diagonal block is
always device-j-tiles 0..3 -- a single uniform SPMD program for all cores.

The squared-norm terms are folded into the Gram matmul itself as a K=4
rank update (ones rows x hi/lo bf16 split of -0.5*|x|^2), so each dist
tile is PE-matmul -> ACT-sqrt -> PE-matmul with no vector op in the loop.
The diagonal is forced to exact zero (reference's clipped diagonal is
~1e-6, negligible): the diagonal block gets -BIG added before the sqrt
(keeping the sqrt argument positive) and a (1 - I) multiply after it.

Precision: the big Gram matmul runs in bf16 (full PE rate; error on loss2
~1e-4 relative), the small [B, C] table in fp32 (loss1/prec decisions).
"""

import numpy as np
import ml_dtypes
from contextlib import ExitStack

import jax
import concourse.bass as bass
import concourse.tile as tile
import concourse.mybir as mybir
from concourse import bacc
from concourse.bass2jax import install_neuronx_cc_hook, _bass_exec_p, partition_id_tensor

B, D, C = 4096, 512, 100
N_CORES = 8
P = 128
SH = B // N_CORES          # 512 rows per core
NJ = B // P                # 32 j-tiles
KT = D // P                # 4 k-tiles
NCH = SH // P              # 4 i-chunks per core
CCW = 1024                 # xt column-chunk width (for DMA/compute overlap)
NCC = B // CCW             # 4 column chunks
BIG = 1.0e12
IAML_MARGIN = 5.0

f32 = mybir.dt.float32
bf16 = mybir.dt.bfloat16
BF16_NP = ml_dtypes.bfloat16

Alu = mybir.AluOpType
Act = mybir.ActivationFunctionType


def _build_program():
    nc = bacc.Bacc("TRN2", target_bir_lowering=False, debug=False,
                   enable_asserts=True, num_devices=1)

    # ---- DRAM I/O (per core; host pre-rotates the j axis by the shard offset)
    xt_d = nc.dram_tensor("xt", [D, B], bf16, kind="ExternalInput")       # X^T, cols rotated
    xe_d = nc.dram_tensor("xe", [4, B], bf16, kind="ExternalInput")       # [hi;lo;1;1] j-side
    xr_d = nc.dram_tensor("xr", [4, SH], bf16, kind="ExternalInput")      # [1;1;hi;lo] i-side
    xst_d = nc.dram_tensor("xst", [D, SH], f32, kind="ExternalInput")     # X_shard^T (fp32)
    ct_d = nc.dram_tensor("ct", [D, C], f32, kind="ExternalInput")        # centers^T
    xa_d = nc.dram_tensor("xa", [2, SH], f32, kind="ExternalInput")       # [-0.5|x|^2; 1] i-side
    xb_d = nc.dram_tensor("xb", [2, C], f32, kind="ExternalInput")        # [1; -0.5(|c|^2+BIGabs)]
    ohp_d = nc.dram_tensor("ohp", [P, NJ * C], bf16, kind="ExternalInput")  # one-hot, j-tiled+rotated
    ohs_d = nc.dram_tensor("ohs", [SH, C], f32, kind="ExternalInput")     # one-hot shard rows
    ohsb_d = nc.dram_tensor("ohsb", [SH, C], f32, kind="ExternalInput")   # BIG * one-hot shard rows
    nI_d = nc.dram_tensor("nI", [P, P], f32, kind="ExternalInput")        # -BIG*I
    cI_d = nc.dram_tensor("cI", [P, P], bf16, kind="ExternalInput")       # 1 - I
    out_d = nc.dram_tensor("out", [P, 2 * NCH], f32, kind="ExternalOutput")
    out2_d = nc.dram_tensor("out2", [C, SH], f32, kind="ExternalOutput")  # S^T

    with tile.TileContext(nc) as tc, ExitStack() as ctx:
        singles = ctx.enter_context(tc.tile_pool(name="singles", bufs=1))
        tmp = ctx.enter_context(tc.tile_pool(name="tmp", bufs=2))
        dpool = ctx.enter_context(tc.tile_pool(name="dist", bufs=3))
        gram_pool = ctx.enter_context(tc.tile_pool(name="gram", bufs=3, space="PSUM"))
        g1_pool = ctx.enter_context(tc.tile_pool(name="g1", bufs=2, space="PSUM"))
        s_pool = ctx.enter_context(tc.tile_pool(name="sacc", bufs=1, space="PSUM"))

        # ---- resident SBUF tiles (small inputs first: phase A starts early)
        xst_sb = []
        for k in range(KT):
            t_ = singles.tile([P, SH], f32, tag=f"xst{k}")
            nc.sync.dma_start(out=t_, in_=xst_d.ap()[k * P:(k + 1) * P, :])
            xst_sb.append(t_)
        ct_sb = []
        for k in range(KT):
            t_ = singles.tile([P, C], f32, tag=f"ct{k}")
            nc.sync.dma_start(out=t_, in_=ct_d.ap()[k * P:(k + 1) * P, :])
            ct_sb.append(t_)
        xa_sb = singles.tile([2, SH], f32, tag="xa")
        nc.sync.dma_start(out=xa_sb, in_=xa_d.ap())
        xb_sb = singles.tile([2, C], f32, tag="xb")
        nc.sync.dma_start(out=xb_sb, in_=xb_d.ap())
        ohs_sb, ohsb_sb = [], []
        for k in range(NCH):
            t_ = singles.tile([P, C], f32, tag=f"ohs{k}")
            nc.sync.dma_start(out=t_, in_=ohs_d.ap()[k * P:(k + 1) * P, :])
            ohs_sb.append(t_)
            t_ = singles.tile([P, C], f32, tag=f"ohsb{k}")
            nc.sync.dma_start(out=t_, in_=ohsb_d.ap()[k * P:(k + 1) * P, :])
            ohsb_sb.append(t_)
        xe_sb = singles.tile([4, B], bf16, tag="xe")
        nc.sync.dma_start(out=xe_sb, in_=xe_d.ap())
        xr_sb = singles.tile([4, SH], bf16, tag="xr")
        nc.sync.dma_start(out=xr_sb, in_=xr_d.ap())
        nI_sb = singles.tile([P, P], f32, tag="nI")
        nc.sync.dma_start(out=nI_sb, in_=nI_d.ap())
        cI_sb = singles.tile([P, P], bf16, tag="cI")
        nc.sync.dma_start(out=cI_sb, in_=cI_d.ap())

        # big inputs: xt in (k-tile x column-chunk) pieces so early j-tiles
        # can start while the tail still streams in
        xt_sb = [[None] * NCC for _ in range(KT)]
        for cc in range(NCC):
            for k in range(KT):
                t_ = singles.tile([P, CCW], bf16, tag=f"xt{k}_{cc}")
                nc.sync.dma_start(
                    out=t_,
                    in_=xt_d.ap()[k * P:(k + 1) * P, cc * CCW:(cc + 1) * CCW])
                xt_sb[k][cc] = t_
        ohp_sb = singles.tile([P, NJ * C], bf16, tag="ohp")
        nc.sync.dma_start(out=ohp_sb, in_=ohp_d.ap())

        out_sb = singles.tile([P, 2 * NCH], f32, tag="out")

        # ---- phase A: [B, C] center-distance table -> loss1 terms + prec
        for k in range(NCH):
            g1 = g1_pool.tile([P, C], f32)
            for kt in range(KT):
                nc.tensor.matmul(g1, xst_sb[kt][:, k * P:(k + 1) * P], ct_sb[kt],
                                 start=(kt == 0), stop=False)
            nc.tensor.matmul(g1, xa_sb[:, k * P:(k + 1) * P], xb_sb,
                             start=False, stop=True)
            d2 = tmp.tile([P, C], f32, tag="d2")
            nc.scalar.activation(out=d2, in_=g1, func=Act.Sqrt,
                                 bias=0.0, scale=-2.0)
            jk = tmp.tile([P, C], f32, tag="jk")
            ap = tmp.tile([P, 1], f32, tag="ap")
            nc.vector.scalar_tensor_tensor(out=jk, in0=d2, scalar=1.0,
                                           in1=ohs_sb[k], op0=Alu.mult,
                                           op1=Alu.mult, accum_out=ap)
            jk2 = tmp.tile([P, C], f32, tag="jk2")
            an = tmp.tile([P, 1], f32, tag="an")
            nc.vector.tensor_add(jk2, d2, ohsb_sb[k])
            nc.vector.tensor_reduce(an, jk2, axis=mybir.AxisListType.X, op=Alu.min)
            diff = tmp.tile([P, 1], f32, tag="diff")
            nc.vector.tensor_sub(diff, ap, an)
            nc.vector.tensor_scalar_max(out_sb[:, 2 * k:2 * k + 1], diff, 0.0)
            nc.vector.tensor_tensor(out=out_sb[:, 2 * k + 1:2 * k + 2],
                                    in0=an, in1=ap, op=Alu.is_gt)

        # ---- phase B: pdist(x, x) tiles + per-class row sums S^T
        st_acc = s_pool.tile([C, SH], f32, tag="st", name="st_acc")
        for t in range(NJ):
            cc, col0 = (t * P) // CCW, (t * P) % CCW
            gram = gram_pool.tile([P, SH], f32)
            for kt in range(KT):
                nc.tensor.matmul(gram, xt_sb[kt][cc][:, col0:col0 + P],
                                 xt_sb[kt][0][:, 0:SH],
                                 start=(kt == 0), stop=False)
            # rank-4 update folds in -0.5(|x_j|^2 + |x_i|^2)
            nc.tensor.matmul(gram, xe_sb[:, t * P:(t + 1) * P], xr_sb,
                             start=False, stop=True)
            dist = dpool.tile([P, SH], bf16)
            nc.scalar.activation(out=dist, in_=gram, func=Act.Sqrt,
                                 bias=0.0, scale=-2.0)
            if t < NCH:
                # diagonal block: recompute with -BIG on the diagonal
                # (positive sqrt argument), then zero the diagonal exactly
                fx = tmp.tile([P, P], f32, tag="fx")
                nc.vector.tensor_add(fx, gram[:, t * P:(t + 1) * P], nI_sb)
                nc.scalar.activation(out=dist[:, t * P:(t + 1) * P], in_=fx,
                                     func=Act.Sqrt, bias=0.0, scale=-2.0)
                nc.vector.tensor_tensor(out=dist[:, t * P:(t + 1) * P],
                                        in0=dist[:, t * P:(t + 1) * P],
                                        in1=cI_sb, op=Alu.mult)
            nc.tensor.matmul(st_acc, ohp_sb[:, t * C:(t + 1) * C], dist,
                             start=(t == 0), stop=(t == NJ - 1),
                             skip_group_check=True)

        # ---- tail: ship S^T; host finishes loss2
        sts = singles.tile([C, SH], f32, tag="sts")
        nc.vector.tensor_copy(sts, st_acc)
        nc.sync.dma_start(out=out2_d.ap(), in_=sts)
        nc.sync.dma_start(out=out_d.ap(), in_=out_sb)

    nc.compile()
    return nc


_RUNNER = None


def _make_runner():
    """Build the program once and return a cached callable
    in_maps -> list of per-core {"out": ..., "out2": ...}. Mirrors
    concourse.bass2jax.run_bass_via_pjrt but keeps the jitted executable
    alive so repeated kernel() calls don't recompile."""
    from jax.sharding import Mesh, PartitionSpec
    from jax.experimental.shard_map import shard_map

    nc = _build_program()
    install_neuronx_cc_hook()

    partition_name = nc.partition_id_tensor.name if nc.partition_id_tensor else None
    in_names, out_names, out_avals, zero_shapes = [], [], [], []
    for alloc in nc.m.functions[0].allocations:
        if not isinstance(alloc, mybir.MemoryLocationSet):
            continue
        name = alloc.memorylocations[0].name
        if alloc.kind == "ExternalInput":
            if name != partition_name:
                in_names.append(name)
        elif alloc.kind == "ExternalOutput":
            shape = tuple(alloc.tensor_shape)
            dtype = mybir.dt.np(alloc.dtype)
            out_names.append(name)
            out_avals.append(jax.core.ShapedArray(shape, dtype))
            zero_shapes.append((shape, dtype))
    n_params = len(in_names)
    n_outs = len(out_avals)
    all_in_names = list(in_names) + list(out_names)
    if partition_name is not None:
        all_in_names.append(partition_name)
    donate = tuple(range(n_params, n_params + n_outs))

    def _body(*args):
        operands = list(args)
        if partition_name is not None:
            operands.append(partition_id_tensor())
        outs = _bass_exec_p.bind(
            *operands,
            out_avals=tuple(out_avals),
            in_names=tuple(all_in_names),
            out_names=tuple(out_names),
            lowering_input_output_aliases=(),
            sim_require_finite=True,
            sim_require_nnan=True,
            nc=nc,
        )
        return tuple(outs)

    devices = jax.devices()[:N_CORES]
    mesh = Mesh(np.asarray(devices), ("core",))
    in_specs = (PartitionSpec("core"),) * (n_params + n_outs)
    out_specs = (PartitionSpec("core"),) * n_outs
    sharded = jax.jit(
        shard_map(_body, mesh=mesh, in_specs=in_specs, out_specs=out_specs,
                  check_rep=False),
        donate_argnums=donate, keep_unused=True)

    def run(in_maps):
        concat_in = [
            np.concatenate([np.asarray(in_maps[c][name]) for c in range(N_CORES)],
                           axis=0)
            for name in in_names
        ]
        concat_zeros = [np.zeros((N_CORES * s[0], *s[1:]), dt)
                        for (s, dt) in zero_shapes]
        out_arrs = sharded(*concat_in, *concat_zeros)
        return [
            {name: np.asarray(out_arrs[i]).reshape(N_CORES, *out_avals[i].shape)[c]
             for i, name in enumerate(out_names)}
            for c in range(N_CORES)
        ]

    return run


def _get_runner():
    global _RUNNER
    if _RUNNER is None:
        _RUNNER = _make_runner()
    return _RUNNER


def _hilo(v):
    """Split fp32 vector v into bf16 hi/lo with hi+lo ~ v (double-bf16)."""
    hi = v.astype(BF16_NP)
    lo = (v - hi.astype(np.float32)).astype(BF16_NP)
    return hi, lo


def make_in_maps(inputs, targets, centers):
    x = np.ascontiguousarray(np.asarray(inputs, dtype=np.float32))
    t = np.asarray(targets).astype(np.int64)
    c = np.ascontiguousarray(np.asarray(centers, dtype=np.float32))

    sqx = np.sum(x * x, axis=1, dtype=np.float32)          # [B]
    sqc = np.sum(c * c, axis=1, dtype=np.float32)          # [C]
    cnt = np.bincount(t, minlength=C).astype(np.float32)   # [C]
    absent = (cnt == 0).astype(np.float32)
    onehot = (t[:, None] == np.arange(C)[None, :]).astype(np.float32)  # [B, C]

    xtT = np.ascontiguousarray(x.T)                        # [D, B]
    ctT = np.ascontiguousarray(c.T)                        # [D, C]
    nI = (-BIG) * np.eye(P, dtype=np.float32)
    cI = (1.0 - np.eye(P)).astype(BF16_NP)
    ones_b = np.ones(B, dtype=np.float32)

    in_maps = []
    for core in range(N_CORES):
        off = core * SH
        xr_cols = np.roll(xtT, -off, axis=1)
        ohr = np.roll(onehot, -off, axis=0)
        sqxr = np.roll(sqx, -off)
        hi_j, lo_j = _hilo(-0.5 * sqxr)
        hi_i, lo_i = _hilo(-0.5 * sqx[off:off + SH])
        xe = np.stack([hi_j, lo_j, ones_b[:B].astype(BF16_NP),
                       ones_b[:B].astype(BF16_NP)])          # [4, B]
        xr_ = np.stack([np.ones(SH, BF16_NP), np.ones(SH, BF16_NP),
                        hi_i, lo_i])                          # [4, SH]
        xa = np.stack([-0.5 * sqx[off:off + SH],
                       np.ones(SH, np.float32)])              # [2, SH]
        xb = np.stack([np.ones(C, np.float32),
                       -0.5 * (sqc + BIG * absent)])          # [2, C]
        in_maps.append({
            "xt": xr_cols.astype(BF16_NP),
            "xe": np.ascontiguousarray(xe),
            "xr": np.ascontiguousarray(xr_),
            "xst": np.ascontiguousarray(xtT[:, off:off + SH]),
            "ct": ctT,
            "xa": np.ascontiguousarray(xa),
            "xb": np.ascontiguousarray(xb),
            "ohp": np.ascontiguousarray(
                ohr.reshape(NJ, P, C).transpose(1, 0, 2).reshape(P, NJ * C)
            ).astype(BF16_NP),
            "ohs": np.ascontiguousarray(onehot[off:off + SH]),
            "ohsb": np.ascontiguousarray(BIG * onehot[off:off + SH]),
            "nI": nI,
            "cI": cI,
        })
    return in_maps


def finish(targets, per_core_out, per_core_out2):
    t = np.asarray(targets).astype(np.int64)
    cnt = np.bincount(t, minlength=C).astype(np.float64)

    outs = np.stack(per_core_out).astype(np.float64)       # [8, 128, 8]
    l1 = outs[:, :, 0::2].sum()
    pr = outs[:, :, 1::2].sum()

    st = np.stack(per_core_out2).astype(np.float64)        # [8, C, SH]
    st_full = st.transpose(0, 2, 1).reshape(B, C)          # [B, C] = S
    pos_sum = st_full[np.arange(B), t]
    tot_sum = st_full.sum(axis=1)
    pos_cnt = cnt[t]
    pos_mean = pos_sum / pos_cnt
    neg_mean = (tot_sum - pos_sum) / (B - pos_cnt)
    l2 = (pos_mean + np.maximum(IAML_MARGIN - neg_mean, 0.0)).sum()

    loss = np.float32(l1 / B + 0.5 * (l2 / B))
    prec = np.float32(pr / B)
    return (np.asarray(loss, dtype=np.float32), np.asarray(prec, dtype=np.float32))


def kernel(inputs, targets, centers):
    in_maps = make_in_maps(inputs, targets, centers)
    results = _get_runner()(in_maps)
    return finish(targets,
                  [results[i]["out"] for i in range(N_CORES)],
                  [results[i]["out2"] for i in range(N_CORES)])


# revision 14
# speedup vs baseline: 1.1782x; 1.1782x over previous
"""Trainium2 Bass kernel for CompactCenterLoss (B=4096, D=512, C=100, 8 cores).

Math notes (vs the reference):
  dist[i, j] = ||x_i - centers[t_j]|| depends on j only through the class
  t_j, so the first BxB table collapses to a [B, C] table D2:
      dist_ap[i] = D2[i, t_i]                      (all same-class j equal)
      dist_an[i] = min_{c present, c != t_i} D2[i, c]
  Only pdist(x, x) needs the full BxB compute. Its masked row sums are
  obtained per class via a matmul with the one-hot matrix O [B, C]:
      S^T[c, i] = sum_j O[j, c] * dist[j, i]
      pos_sum_i = S^T[t_i, i],  tot_sum_i = sum_c S^T[c, i]

Sharding: batch rows are split across 8 cores (512 rows each). Every core
computes dist^T tiles [128 j x 512 i_shard] against the full (replicated)
input, using a per-core ROTATION of the j axis so the diagonal block is
always device-j-tiles 0..3 -- a single uniform SPMD program for all cores.

The squared-norm terms are folded into the Gram matmul itself as a K=4
rank update (ones rows x hi/lo bf16 split of -0.5*|x|^2), so each dist
tile is PE-matmul -> ACT-sqrt -> PE-matmul with no vector op in the loop.
The diagonal is forced to exact zero (reference's clipped diagonal is
~1e-6, negligible): the diagonal block gets -BIG added before the sqrt
(keeping the sqrt argument positive) and a (1 - I) multiply after it.

Precision: the big Gram matmul runs in bf16 (full PE rate; error on loss2
~1e-4 relative), the small [B, C] table in fp32 (loss1/prec decisions).
"""

import numpy as np
import ml_dtypes
from contextlib import ExitStack

import jax
import concourse.bass as bass
import concourse.tile as tile
import concourse.mybir as mybir
from concourse import bacc
from concourse.bass2jax import install_neuronx_cc_hook, _bass_exec_p, partition_id_tensor

B, D, C = 4096, 512, 100
N_CORES = 8
P = 128
SH = B // N_CORES          # 512 rows per core
NJ = B // P                # 32 j-tiles
KT = D // P                # 4 k-tiles
NCH = SH // P              # 4 i-chunks per core
CCW = 1024                 # xt column-chunk width (for DMA/compute overlap)
NCC = B // CCW             # 4 column chunks
BIG = 1.0e12
IAML_MARGIN = 5.0

f32 = mybir.dt.float32
bf16 = mybir.dt.bfloat16
BF16_NP = ml_dtypes.bfloat16

Alu = mybir.AluOpType
Act = mybir.ActivationFunctionType


def _build_program():
    nc = bacc.Bacc("TRN2", target_bir_lowering=False, debug=False,
                   enable_asserts=True, num_devices=1)

    # ---- DRAM I/O (per core; host pre-rotates the j axis by the shard offset)
    xt_d = nc.dram_tensor("xt", [D, B], bf16, kind="ExternalInput")       # X^T, cols rotated
    sqxib_d = nc.dram_tensor("sqxib", [P, SH], f32, kind="ExternalInput")  # |x_i|^2 bcast over partitions
    sqxc_d = nc.dram_tensor("sqxc", [P, NJ], f32, kind="ExternalInput")   # |x_j|^2 per j-tile col (rotated)
    xst_d = nc.dram_tensor("xst", [D, SH], f32, kind="ExternalInput")     # X_shard^T (fp32)
    ct_d = nc.dram_tensor("ct", [D, C], f32, kind="ExternalInput")        # centers^T
    xa_d = nc.dram_tensor("xa", [2, SH], f32, kind="ExternalInput")       # [-0.5|x|^2; 1] i-side
    xb_d = nc.dram_tensor("xb", [2, C], f32, kind="ExternalInput")        # [1; -0.5(|c|^2+BIGabs)]
    ohp_d = nc.dram_tensor("ohp", [P, NJ * C], bf16, kind="ExternalInput")  # one-hot, j-tiled+rotated
    ohs_d = nc.dram_tensor("ohs", [SH, C], f32, kind="ExternalInput")     # one-hot shard rows
    ohsb_d = nc.dram_tensor("ohsb", [SH, C], f32, kind="ExternalInput")   # BIG * one-hot shard rows
    pI_d = nc.dram_tensor("pI", [P, P], f32, kind="ExternalInput")        # +2*BIG*I
    cI_d = nc.dram_tensor("cI", [P, P], bf16, kind="ExternalInput")       # 1 - I
    out_d = nc.dram_tensor("out", [P, 2 * NCH], f32, kind="ExternalOutput")
    out2_d = nc.dram_tensor("out2", [C, SH], f32, kind="ExternalOutput")  # S^T

    with tile.TileContext(nc) as tc, ExitStack() as ctx:
        singles = ctx.enter_context(tc.tile_pool(name="singles", bufs=1))
        tmp = ctx.enter_context(tc.tile_pool(name="tmp", bufs=2))
        dpool = ctx.enter_context(tc.tile_pool(name="dist", bufs=4))
        spool = ctx.enter_context(tc.tile_pool(name="sq", bufs=3))
        gram_pool = ctx.enter_context(tc.tile_pool(name="gram", bufs=3, space="PSUM"))
        g1_pool = ctx.enter_context(tc.tile_pool(name="g1", bufs=2, space="PSUM"))
        s_pool = ctx.enter_context(tc.tile_pool(name="sacc", bufs=1, space="PSUM"))

        # ---- resident SBUF tiles (small inputs first: phase A starts early)
        xst_sb = []
        for k in range(KT):
            t_ = singles.tile([P, SH], f32, tag=f"xst{k}")
            nc.sync.dma_start(out=t_, in_=xst_d.ap()[k * P:(k + 1) * P, :])
            xst_sb.append(t_)
        ct_sb = []
        for k in range(KT):
            t_ = singles.tile([P, C], f32, tag=f"ct{k}")
            nc.sync.dma_start(out=t_, in_=ct_d.ap()[k * P:(k + 1) * P, :])
            ct_sb.append(t_)
        xa_sb = singles.tile([2, SH], f32, tag="xa")
        nc.sync.dma_start(out=xa_sb, in_=xa_d.ap())
        xb_sb = singles.tile([2, C], f32, tag="xb")
        nc.sync.dma_start(out=xb_sb, in_=xb_d.ap())
        ohs_sb, ohsb_sb = [], []
        for k in range(NCH):
            t_ = singles.tile([P, C], f32, tag=f"ohs{k}")
            nc.sync.dma_start(out=t_, in_=ohs_d.ap()[k * P:(k + 1) * P, :])
            ohs_sb.append(t_)
            t_ = singles.tile([P, C], f32, tag=f"ohsb{k}")
            nc.sync.dma_start(out=t_, in_=ohsb_d.ap()[k * P:(k + 1) * P, :])
            ohsb_sb.append(t_)
        sqxib_sb = singles.tile([P, SH], f32, tag="sqxib")
        nc.sync.dma_start(out=sqxib_sb, in_=sqxib_d.ap())
        sqxc_sb = singles.tile([P, NJ], f32, tag="sqxc")
        nc.sync.dma_start(out=sqxc_sb, in_=sqxc_d.ap())
        pI_sb = singles.tile([P, P], f32, tag="pI")
        nc.sync.dma_start(out=pI_sb, in_=pI_d.ap())
        cI_sb = singles.tile([P, P], bf16, tag="cI")
        nc.sync.dma_start(out=cI_sb, in_=cI_d.ap())

        # big inputs: xt in (k-tile x column-chunk) pieces so early j-tiles
        # can start while the tail still streams in
        xt_sb = [[None] * NCC for _ in range(KT)]
        for cc in range(NCC):
            for k in range(KT):
                t_ = singles.tile([P, CCW], bf16, tag=f"xt{k}_{cc}")
                nc.sync.dma_start(
                    out=t_,
                    in_=xt_d.ap()[k * P:(k + 1) * P, cc * CCW:(cc + 1) * CCW])
                xt_sb[k][cc] = t_
        ohp_sb = singles.tile([P, NJ * C], bf16, tag="ohp")
        nc.sync.dma_start(out=ohp_sb, in_=ohp_d.ap())

        out_sb = singles.tile([P, 2 * NCH], f32, tag="out")

        # ---- phase A: [B, C] center-distance table -> loss1 terms + prec
        for k in range(NCH):
            g1 = g1_pool.tile([P, C], f32)
            for kt in range(KT):
                nc.tensor.matmul(g1, xst_sb[kt][:, k * P:(k + 1) * P], ct_sb[kt],
                                 start=(kt == 0), stop=False)
            nc.tensor.matmul(g1, xa_sb[:, k * P:(k + 1) * P], xb_sb,
                             start=False, stop=True)
            d2 = tmp.tile([P, C], f32, tag="d2")
            nc.scalar.activation(out=d2, in_=g1, func=Act.Sqrt,
                                 bias=0.0, scale=-2.0)
            jk = tmp.tile([P, C], f32, tag="jk")
            ap = tmp.tile([P, 1], f32, tag="ap")
            nc.vector.scalar_tensor_tensor(out=jk, in0=d2, scalar=1.0,
                                           in1=ohs_sb[k], op0=Alu.mult,
                                           op1=Alu.mult, accum_out=ap)
            jk2 = tmp.tile([P, C], f32, tag="jk2")
            an = tmp.tile([P, 1], f32, tag="an")
            nc.vector.tensor_add(jk2, d2, ohsb_sb[k])
            nc.vector.tensor_reduce(an, jk2, axis=mybir.AxisListType.X, op=Alu.min)
            diff = tmp.tile([P, 1], f32, tag="diff")
            nc.vector.tensor_sub(diff, ap, an)
            nc.vector.tensor_scalar_max(out_sb[:, 2 * k:2 * k + 1], diff, 0.0)
            nc.vector.tensor_tensor(out=out_sb[:, 2 * k + 1:2 * k + 2],
                                    in0=an, in1=ap, op=Alu.is_gt)

        # ---- phase B: pdist(x, x) tiles + per-class row sums S^T
        # The S matmul for tile t is emitted two tiles late so the in-order
        # PE never waits on the DVE->ACT sqrt chain of its own tile.
        st_acc = s_pool.tile([C, SH], f32, tag="st", name="st_acc")
        dists = [None] * NJ

        def s_matmul(u):
            nc.tensor.matmul(st_acc, ohp_sb[:, u * C:(u + 1) * C], dists[u],
                             start=(u == 0), stop=(u == NJ - 1),
                             skip_group_check=True)

        for t in range(NJ):
            cc, col0 = (t * P) // CCW, (t * P) % CCW
            gram = gram_pool.tile([P, SH], f32)
            for kt in range(KT):
                nc.tensor.matmul(gram, xt_sb[kt][cc][:, col0:col0 + P],
                                 xt_sb[kt][0][:, 0:SH],
                                 start=(kt == 0), stop=(kt == KT - 1))
            if t >= 2:
                s_matmul(t - 2)
            # sq = -2*gram + |x_i|^2 ; dist = sqrt(sq + |x_j|^2)
            sq = spool.tile([P, SH], f32)
            nc.vector.scalar_tensor_tensor(out=sq, in0=gram, scalar=-2.0,
                                           in1=sqxib_sb, op0=Alu.mult, op1=Alu.add)
            dist = dpool.tile([P, SH], bf16)
            nc.scalar.activation(out=dist, in_=sq, func=Act.Sqrt,
                                 bias=sqxc_sb[:, t:t + 1], scale=1.0)
            if t < NCH:
                # diagonal block: recompute with +2*BIG on the diagonal
                # (positive sqrt argument), then zero the diagonal exactly
                fx = tmp.tile([P, P], f32, tag="fx")
                nc.vector.tensor_add(fx, sq[:, t * P:(t + 1) * P], pI_sb)
                nc.scalar.activation(out=dist[:, t * P:(t + 1) * P], in_=fx,
                                     func=Act.Sqrt, bias=sqxc_sb[:, t:t + 1],
                                     scale=1.0)
                nc.vector.tensor_tensor(out=dist[:, t * P:(t + 1) * P],
                                        in0=dist[:, t * P:(t + 1) * P],
                                        in1=cI_sb, op=Alu.mult)
            dists[t] = dist
        s_matmul(NJ - 2)
        s_matmul(NJ - 1)

        # ---- tail: ship S^T; host finishes loss2
        sts = singles.tile([C, SH], f32, tag="sts")
        nc.vector.tensor_copy(sts, st_acc)
        nc.sync.dma_start(out=out2_d.ap(), in_=sts)
        nc.sync.dma_start(out=out_d.ap(), in_=out_sb)

    nc.compile()
    return nc


_RUNNER = None


def _make_runner():
    """Build the program once and return a cached callable
    in_maps -> list of per-core {"out": ..., "out2": ...}. Mirrors
    concourse.bass2jax.run_bass_via_pjrt but keeps the jitted executable
    alive so repeated kernel() calls don't recompile."""
    from jax.sharding import Mesh, PartitionSpec
    from jax.experimental.shard_map import shard_map

    nc = _build_program()
    install_neuronx_cc_hook()

    partition_name = nc.partition_id_tensor.name if nc.partition_id_tensor else None
    in_names, out_names, out_avals, zero_shapes = [], [], [], []
    for alloc in nc.m.functions[0].allocations:
        if not isinstance(alloc, mybir.MemoryLocationSet):
            continue
        name = alloc.memorylocations[0].name
        if alloc.kind == "ExternalInput":
            if name != partition_name:
                in_names.append(name)
        elif alloc.kind == "ExternalOutput":
            shape = tuple(alloc.tensor_shape)
            dtype = mybir.dt.np(alloc.dtype)
            out_names.append(name)
            out_avals.append(jax.core.ShapedArray(shape, dtype))
            zero_shapes.append((shape, dtype))
    n_params = len(in_names)
    n_outs = len(out_avals)
    all_in_names = list(in_names) + list(out_names)
    if partition_name is not None:
        all_in_names.append(partition_name)
    donate = tuple(range(n_params, n_params + n_outs))

    def _body(*args):
        operands = list(args)
        if partition_name is not None:
            operands.append(partition_id_tensor())
        outs = _bass_exec_p.bind(
            *operands,
            out_avals=tuple(out_avals),
            in_names=tuple(all_in_names),
            out_names=tuple(out_names),
            lowering_input_output_aliases=(),
            sim_require_finite=True,
            sim_require_nnan=True,
            nc=nc,
        )
        return tuple(outs)

    devices = jax.devices()[:N_CORES]
    mesh = Mesh(np.asarray(devices), ("core",))
    in_specs = (PartitionSpec("core"),) * (n_params + n_outs)
    out_specs = (PartitionSpec("core"),) * n_outs
    sharded = jax.jit(
        shard_map(_body, mesh=mesh, in_specs=in_specs, out_specs=out_specs,
                  check_rep=False),
        donate_argnums=donate, keep_unused=True)

    def run(in_maps):
        concat_in = [
            np.concatenate([np.asarray(in_maps[c][name]) for c in range(N_CORES)],
                           axis=0)
            for name in in_names
        ]
        concat_zeros = [np.zeros((N_CORES * s[0], *s[1:]), dt)
                        for (s, dt) in zero_shapes]
        out_arrs = sharded(*concat_in, *concat_zeros)
        return [
            {name: np.asarray(out_arrs[i]).reshape(N_CORES, *out_avals[i].shape)[c]
             for i, name in enumerate(out_names)}
            for c in range(N_CORES)
        ]

    return run


def _get_runner():
    global _RUNNER
    if _RUNNER is None:
        _RUNNER = _make_runner()
    return _RUNNER


def _hilo(v):
    """Split fp32 vector v into bf16 hi/lo with hi+lo ~ v (double-bf16)."""
    hi = v.astype(BF16_NP)
    lo = (v - hi.astype(np.float32)).astype(BF16_NP)
    return hi, lo


def make_in_maps(inputs, targets, centers):
    x = np.ascontiguousarray(np.asarray(inputs, dtype=np.float32))
    t = np.asarray(targets).astype(np.int64)
    c = np.ascontiguousarray(np.asarray(centers, dtype=np.float32))

    sqx = np.sum(x * x, axis=1, dtype=np.float32)          # [B]
    sqc = np.sum(c * c, axis=1, dtype=np.float32)          # [C]
    cnt = np.bincount(t, minlength=C).astype(np.float32)   # [C]
    absent = (cnt == 0).astype(np.float32)
    onehot = (t[:, None] == np.arange(C)[None, :]).astype(np.float32)  # [B, C]

    xtT = np.ascontiguousarray(x.T)                        # [D, B]
    ctT = np.ascontiguousarray(c.T)                        # [D, C]
    pI = (2.0 * BIG) * np.eye(P, dtype=np.float32)
    cI = (1.0 - np.eye(P)).astype(BF16_NP)

    in_maps = []
    for core in range(N_CORES):
        off = core * SH
        xr_cols = np.roll(xtT, -off, axis=1)
        ohr = np.roll(onehot, -off, axis=0)
        sqxr = np.roll(sqx, -off)
        xa = np.stack([-0.5 * sqx[off:off + SH],
                       np.ones(SH, np.float32)])              # [2, SH]
        xb = np.stack([np.ones(C, np.float32),
                       -0.5 * (sqc + BIG * absent)])          # [2, C]
        in_maps.append({
            "xt": xr_cols.astype(BF16_NP),
            "sqxib": np.tile(sqx[off:off + SH][None, :], (P, 1)),
            "sqxc": np.ascontiguousarray(sqxr.reshape(NJ, P).T),
            "xst": np.ascontiguousarray(xtT[:, off:off + SH]),
            "ct": ctT,
            "xa": np.ascontiguousarray(xa),
            "xb": np.ascontiguousarray(xb),
            "ohp": np.ascontiguousarray(
                ohr.reshape(NJ, P, C).transpose(1, 0, 2).reshape(P, NJ * C)
            ).astype(BF16_NP),
            "ohs": np.ascontiguousarray(onehot[off:off + SH]),
            "ohsb": np.ascontiguousarray(BIG * onehot[off:off + SH]),
            "pI": pI,
            "cI": cI,
        })
    return in_maps


def finish(targets, per_core_out, per_core_out2):
    t = np.asarray(targets).astype(np.int64)
    cnt = np.bincount(t, minlength=C).astype(np.float64)

    outs = np.stack(per_core_out).astype(np.float64)       # [8, 128, 8]
    l1 = outs[:, :, 0::2].sum()
    pr = outs[:, :, 1::2].sum()

    st = np.stack(per_core_out2).astype(np.float64)        # [8, C, SH]
    st_full = st.transpose(0, 2, 1).reshape(B, C)          # [B, C] = S
    pos_sum = st_full[np.arange(B), t]
    tot_sum = st_full.sum(axis=1)
    pos_cnt = cnt[t]
    pos_mean = pos_sum / pos_cnt
    neg_mean = (tot_sum - pos_sum) / (B - pos_cnt)
    l2 = (pos_mean + np.maximum(IAML_MARGIN - neg_mean, 0.0)).sum()

    loss = np.float32(l1 / B + 0.5 * (l2 / B))
    prec = np.float32(pr / B)
    return (np.asarray(loss, dtype=np.float32), np.asarray(prec, dtype=np.float32))


def kernel(inputs, targets, centers):
    in_maps = make_in_maps(inputs, targets, centers)
    results = _get_runner()(in_maps)
    return finish(targets,
                  [results[i]["out"] for i in range(N_CORES)],
                  [results[i]["out2"] for i in range(N_CORES)])


# revision 16
# speedup vs baseline: 1.5078x; 1.2798x over previous
"""Trainium2 Bass kernel for CompactCenterLoss (B=4096, D=512, C=100, 8 cores).

Math notes (vs the reference):
  dist[i, j] = ||x_i - centers[t_j]|| depends on j only through the class
  t_j, so the first BxB table collapses to a [B, C] table D2:
      dist_ap[i] = D2[i, t_i]                      (all same-class j equal)
      dist_an[i] = min_{c present, c != t_i} D2[i, c]
  Only pdist(x, x) needs the full BxB compute. Its masked row sums are
  obtained per class via a matmul with the one-hot matrix O [B, C]:
      S^T[c, i] = sum_j O[j, c] * dist[j, i]
      pos_sum_i = S^T[t_i, i],  tot_sum_i = sum_c S^T[c, i]

Sharding: batch rows are split across 8 cores (512 rows each). Every core
computes dist^T tiles [128 j x 512 i_shard] against the full (replicated)
input, using a per-core ROTATION of the j axis so the diagonal block is
always device-j-tiles 0..3 -- a single uniform SPMD program for all cores.

The squared-norm terms are folded into the Gram matmul itself as a K=4
rank update (ones rows x hi/lo bf16 split of -0.5*|x|^2), so each dist
tile is PE-matmul -> ACT-sqrt -> PE-matmul with no vector op in the loop.
The diagonal is forced to exact zero (reference's clipped diagonal is
~1e-6, negligible): the diagonal block gets -BIG added before the sqrt
(keeping the sqrt argument positive) and a (1 - I) multiply after it.

Precision: the big Gram matmul runs in bf16 (full PE rate; error on loss2
~1e-4 relative), the small [B, C] table in fp32 (loss1/prec decisions).
"""

import numpy as np
import ml_dtypes
from contextlib import ExitStack

import jax
import concourse.bass as bass
import concourse.tile as tile
import concourse.mybir as mybir
from concourse import bacc
from concourse.bass2jax import install_neuronx_cc_hook, _bass_exec_p, partition_id_tensor

B, D, C = 4096, 512, 100
N_CORES = 8
P = 128
SH = B // N_CORES          # 512 rows per core
NJ = B // P                # 32 j-tiles
KT = D // P                # 4 k-tiles
NCH = SH // P              # 4 i-chunks per core
CCW = 1024                 # xt column-chunk width (for DMA/compute overlap)
NCC = B // CCW             # 4 column chunks
BIG = 1.0e12
IAML_MARGIN = 5.0

f32 = mybir.dt.float32
bf16 = mybir.dt.bfloat16
BF16_NP = ml_dtypes.bfloat16

Alu = mybir.AluOpType
Act = mybir.ActivationFunctionType


def _build_program():
    nc = bacc.Bacc("TRN2", target_bir_lowering=False, debug=False,
                   enable_asserts=True, num_devices=1)

    # ---- DRAM I/O (per core; host pre-rotates the j axis by the shard offset)
    xt_d = nc.dram_tensor("xt", [D, B], bf16, kind="ExternalInput")       # X^T, cols rotated
    sqxib_d = nc.dram_tensor("sqxib", [P, SH], f32, kind="ExternalInput")  # |x_i|^2 bcast over partitions
    sqxc_d = nc.dram_tensor("sqxc", [P, NJ], f32, kind="ExternalInput")   # |x_j|^2 per j-tile col (rotated)
    xst_d = nc.dram_tensor("xst", [D, SH], f32, kind="ExternalInput")     # X_shard^T (fp32)
    ct_d = nc.dram_tensor("ct", [D, C], f32, kind="ExternalInput")        # centers^T
    xa_d = nc.dram_tensor("xa", [2, SH], f32, kind="ExternalInput")       # [-0.5|x|^2; 1] i-side
    xb_d = nc.dram_tensor("xb", [2, C], f32, kind="ExternalInput")        # [1; -0.5(|c|^2+BIGabs)]
    ohp_d = nc.dram_tensor("ohp", [P, NJ * C], bf16, kind="ExternalInput")  # one-hot, j-tiled+rotated
    ohs_d = nc.dram_tensor("ohs", [SH, C], f32, kind="ExternalInput")     # one-hot shard rows
    ohsb_d = nc.dram_tensor("ohsb", [SH, C], f32, kind="ExternalInput")   # BIG * one-hot shard rows
    pI_d = nc.dram_tensor("pI", [P, P], f32, kind="ExternalInput")        # +2*BIG*I
    cI_d = nc.dram_tensor("cI", [P, P], bf16, kind="ExternalInput")       # 1 - I
    out_d = nc.dram_tensor("out", [P, 2 * NCH], f32, kind="ExternalOutput")
    out2_d = nc.dram_tensor("out2", [C, SH], f32, kind="ExternalOutput")  # S^T

    with tile.TileContext(nc) as tc, ExitStack() as ctx:
        singles = ctx.enter_context(tc.tile_pool(name="singles", bufs=1))
        tmp = ctx.enter_context(tc.tile_pool(name="tmp", bufs=2))
        dpool = ctx.enter_context(tc.tile_pool(name="dist", bufs=4))
        spool = ctx.enter_context(tc.tile_pool(name="sq", bufs=3))
        gram_pool = ctx.enter_context(tc.tile_pool(name="gram", bufs=3, space="PSUM"))
        g1_pool = ctx.enter_context(tc.tile_pool(name="g1", bufs=2, space="PSUM"))
        s_pool = ctx.enter_context(tc.tile_pool(name="sacc", bufs=1, space="PSUM"))

        # ---- DMAs in consumption order: phase B streams first, phase A
        # (which runs last, on a warm PE) loads in the shadow of phase B.
        xt_sb = [[None] * NCC for _ in range(KT)]
        for k in range(KT):
            t_ = singles.tile([P, CCW], bf16, tag=f"xt{k}_0")
            nc.sync.dma_start(out=t_, in_=xt_d.ap()[k * P:(k + 1) * P, 0:CCW])
            xt_sb[k][0] = t_
        sqxib_sb = singles.tile([P, SH], f32, tag="sqxib")
        nc.sync.dma_start(out=sqxib_sb, in_=sqxib_d.ap())
        sqxc_sb = singles.tile([P, NJ], f32, tag="sqxc")
        nc.sync.dma_start(out=sqxc_sb, in_=sqxc_d.ap())
        pI_sb = singles.tile([P, P], f32, tag="pI")
        nc.sync.dma_start(out=pI_sb, in_=pI_d.ap())
        cI_sb = singles.tile([P, P], bf16, tag="cI")
        nc.sync.dma_start(out=cI_sb, in_=cI_d.ap())
        ohp_sb = singles.tile([P, NJ * C], bf16, tag="ohp")
        nc.sync.dma_start(out=ohp_sb, in_=ohp_d.ap())
        for cc in range(1, NCC):
            for k in range(KT):
                t_ = singles.tile([P, CCW], bf16, tag=f"xt{k}_{cc}")
                nc.sync.dma_start(
                    out=t_,
                    in_=xt_d.ap()[k * P:(k + 1) * P, cc * CCW:(cc + 1) * CCW])
                xt_sb[k][cc] = t_
        # phase A inputs (needed only near the end)
        xst_sb = []
        for k in range(KT):
            t_ = singles.tile([P, SH], f32, tag=f"xst{k}")
            nc.sync.dma_start(out=t_, in_=xst_d.ap()[k * P:(k + 1) * P, :])
            xst_sb.append(t_)
        ct_sb = []
        for k in range(KT):
            t_ = singles.tile([P, C], f32, tag=f"ct{k}")
            nc.sync.dma_start(out=t_, in_=ct_d.ap()[k * P:(k + 1) * P, :])
            ct_sb.append(t_)
        xa_sb = singles.tile([2, SH], f32, tag="xa")
        nc.sync.dma_start(out=xa_sb, in_=xa_d.ap())
        xb_sb = singles.tile([2, C], f32, tag="xb")
        nc.sync.dma_start(out=xb_sb, in_=xb_d.ap())
        ohs_sb, ohsb_sb = [], []
        for k in range(NCH):
            t_ = singles.tile([P, C], f32, tag=f"ohs{k}")
            nc.sync.dma_start(out=t_, in_=ohs_d.ap()[k * P:(k + 1) * P, :])
            ohs_sb.append(t_)
            t_ = singles.tile([P, C], f32, tag=f"ohsb{k}")
            nc.sync.dma_start(out=t_, in_=ohsb_d.ap()[k * P:(k + 1) * P, :])
            ohsb_sb.append(t_)

        out_sb = singles.tile([P, 2 * NCH], f32, tag="out")

        # ---- phase B: pdist(x, x) tiles + per-class row sums S^T
        # The S matmul for tile t is emitted two tiles late so the in-order
        # PE never waits on the DVE->ACT sqrt chain of its own tile.
        st_acc = s_pool.tile([C, SH], f32, tag="st", name="st_acc")
        dists = [None] * NJ

        def s_matmul(u):
            nc.tensor.matmul(st_acc, ohp_sb[:, u * C:(u + 1) * C], dists[u],
                             start=(u == 0), stop=(u == NJ - 1),
                             skip_group_check=True)

        for t in range(NJ):
            cc, col0 = (t * P) // CCW, (t * P) % CCW
            gram = gram_pool.tile([P, SH], f32)
            for kt in range(KT):
                nc.tensor.matmul(gram, xt_sb[kt][cc][:, col0:col0 + P],
                                 xt_sb[kt][0][:, 0:SH],
                                 start=(kt == 0), stop=(kt == KT - 1))
            if t >= 2:
                s_matmul(t - 2)
            # sq = -2*gram + |x_i|^2 ; dist = sqrt(sq + |x_j|^2)
            sq = spool.tile([P, SH], f32)
            nc.vector.scalar_tensor_tensor(out=sq, in0=gram, scalar=-2.0,
                                           in1=sqxib_sb, op0=Alu.mult, op1=Alu.add)
            dist = dpool.tile([P, SH], bf16)
            nc.scalar.activation(out=dist, in_=sq, func=Act.Sqrt,
                                 bias=sqxc_sb[:, t:t + 1], scale=1.0)
            if t < NCH:
                # diagonal block: recompute with +2*BIG on the diagonal
                # (positive sqrt argument), then zero the diagonal exactly
                fx = tmp.tile([P, P], f32, tag="fx")
                nc.vector.tensor_add(fx, sq[:, t * P:(t + 1) * P], pI_sb)
                nc.scalar.activation(out=dist[:, t * P:(t + 1) * P], in_=fx,
                                     func=Act.Sqrt, bias=sqxc_sb[:, t:t + 1],
                                     scale=1.0)
                nc.vector.tensor_tensor(out=dist[:, t * P:(t + 1) * P],
                                        in0=dist[:, t * P:(t + 1) * P],
                                        in1=cI_sb, op=Alu.mult)
            dists[t] = dist
        s_matmul(NJ - 2)

        # ---- phase A (emitted last: PE is warm, and its fp32 matmuls fill
        # the latency of the last dist tile's DVE->ACT chain)
        for k in range(NCH):
            g1 = g1_pool.tile([P, C], f32)
            for kt in range(KT):
                nc.tensor.matmul(g1, xst_sb[kt][:, k * P:(k + 1) * P], ct_sb[kt],
                                 start=(kt == 0), stop=False)
            nc.tensor.matmul(g1, xa_sb[:, k * P:(k + 1) * P], xb_sb,
                             start=False, stop=True)
            d2 = tmp.tile([P, C], f32, tag="d2")
            nc.scalar.activation(out=d2, in_=g1, func=Act.Sqrt,
                                 bias=0.0, scale=-2.0)
            jk = tmp.tile([P, C], f32, tag="jk")
            ap = tmp.tile([P, 1], f32, tag="ap")
            nc.vector.scalar_tensor_tensor(out=jk, in0=d2, scalar=1.0,
                                           in1=ohs_sb[k], op0=Alu.mult,
                                           op1=Alu.mult, accum_out=ap)
            jk2 = tmp.tile([P, C], f32, tag="jk2")
            an = tmp.tile([P, 1], f32, tag="an")
            nc.vector.tensor_add(jk2, d2, ohsb_sb[k])
            nc.vector.tensor_reduce(an, jk2, axis=mybir.AxisListType.X, op=Alu.min)
            diff = tmp.tile([P, 1], f32, tag="diff")
            nc.vector.tensor_sub(diff, ap, an)
            nc.vector.tensor_scalar_max(out_sb[:, 2 * k:2 * k + 1], diff, 0.0)
            nc.vector.tensor_tensor(out=out_sb[:, 2 * k + 1:2 * k + 2],
                                    in0=an, in1=ap, op=Alu.is_gt)

        s_matmul(NJ - 1)

        # ---- tail: ship S^T; host finishes loss2
        sts = singles.tile([C, SH], f32, tag="sts")
        nc.vector.tensor_copy(sts, st_acc)
        nc.sync.dma_start(out=out2_d.ap(), in_=sts)
        nc.sync.dma_start(out=out_d.ap(), in_=out_sb)

    nc.compile()
    return nc


_RUNNER = None


def _make_runner():
    """Build the program once and return a cached callable
    in_maps -> list of per-core {"out": ..., "out2": ...}. Mirrors
    concourse.bass2jax.run_bass_via_pjrt but keeps the jitted executable
    alive so repeated kernel() calls don't recompile."""
    from jax.sharding import Mesh, PartitionSpec
    from jax.experimental.shard_map import shard_map

    nc = _build_program()
    install_neuronx_cc_hook()

    partition_name = nc.partition_id_tensor.name if nc.partition_id_tensor else None
    in_names, out_names, out_avals, zero_shapes = [], [], [], []
    for alloc in nc.m.functions[0].allocations:
        if not isinstance(alloc, mybir.MemoryLocationSet):
            continue
        name = alloc.memorylocations[0].name
        if alloc.kind == "ExternalInput":
            if name != partition_name:
                in_names.append(name)
        elif alloc.kind == "ExternalOutput":
            shape = tuple(alloc.tensor_shape)
            dtype = mybir.dt.np(alloc.dtype)
            out_names.append(name)
            out_avals.append(jax.core.ShapedArray(shape, dtype))
            zero_shapes.append((shape, dtype))
    n_params = len(in_names)
    n_outs = len(out_avals)
    all_in_names = list(in_names) + list(out_names)
    if partition_name is not None:
        all_in_names.append(partition_name)
    donate = tuple(range(n_params, n_params + n_outs))

    def _body(*args):
        operands = list(args)
        if partition_name is not None:
            operands.append(partition_id_tensor())
        outs = _bass_exec_p.bind(
            *operands,
            out_avals=tuple(out_avals),
            in_names=tuple(all_in_names),
            out_names=tuple(out_names),
            lowering_input_output_aliases=(),
            sim_require_finite=True,
            sim_require_nnan=True,
            nc=nc,
        )
        return tuple(outs)

    devices = jax.devices()[:N_CORES]
    mesh = Mesh(np.asarray(devices), ("core",))
    in_specs = (PartitionSpec("core"),) * (n_params + n_outs)
    out_specs = (PartitionSpec("core"),) * n_outs
    sharded = jax.jit(
        shard_map(_body, mesh=mesh, in_specs=in_specs, out_specs=out_specs,
                  check_rep=False),
        donate_argnums=donate, keep_unused=True)

    def run(in_maps):
        concat_in = [
            np.concatenate([np.asarray(in_maps[c][name]) for c in range(N_CORES)],
                           axis=0)
            for name in in_names
        ]
        concat_zeros = [np.zeros((N_CORES * s[0], *s[1:]), dt)
                        for (s, dt) in zero_shapes]
        out_arrs = sharded(*concat_in, *concat_zeros)
        return [
            {name: np.asarray(out_arrs[i]).reshape(N_CORES, *out_avals[i].shape)[c]
             for i, name in enumerate(out_names)}
            for c in range(N_CORES)
        ]

    return run


def _get_runner():
    global _RUNNER
    if _RUNNER is None:
        _RUNNER = _make_runner()
    return _RUNNER


def _hilo(v):
    """Split fp32 vector v into bf16 hi/lo with hi+lo ~ v (double-bf16)."""
    hi = v.astype(BF16_NP)
    lo = (v - hi.astype(np.float32)).astype(BF16_NP)
    return hi, lo


def make_in_maps(inputs, targets, centers):
    x = np.ascontiguousarray(np.asarray(inputs, dtype=np.float32))
    t = np.asarray(targets).astype(np.int64)
    c = np.ascontiguousarray(np.asarray(centers, dtype=np.float32))

    sqx = np.sum(x * x, axis=1, dtype=np.float32)          # [B]
    sqc = np.sum(c * c, axis=1, dtype=np.float32)          # [C]
    cnt = np.bincount(t, minlength=C).astype(np.float32)   # [C]
    absent = (cnt == 0).astype(np.float32)
    onehot = (t[:, None] == np.arange(C)[None, :]).astype(np.float32)  # [B, C]

    xtT = np.ascontiguousarray(x.T)                        # [D, B]
    ctT = np.ascontiguousarray(c.T)                        # [D, C]
    pI = (2.0 * BIG) * np.eye(P, dtype=np.float32)
    cI = (1.0 - np.eye(P)).astype(BF16_NP)

    in_maps = []
    for core in range(N_CORES):
        off = core * SH
        xr_cols = np.roll(xtT, -off, axis=1)
        ohr = np.roll(onehot, -off, axis=0)
        sqxr = np.roll(sqx, -off)
        xa = np.stack([-0.5 * sqx[off:off + SH],
                       np.ones(SH, np.float32)])              # [2, SH]
        xb = np.stack([np.ones(C, np.float32),
                       -0.5 * (sqc + BIG * absent)])          # [2, C]
        in_maps.append({
            "xt": xr_cols.astype(BF16_NP),
            "sqxib": np.tile(sqx[off:off + SH][None, :], (P, 1)),
            "sqxc": np.ascontiguousarray(sqxr.reshape(NJ, P).T),
            "xst": np.ascontiguousarray(xtT[:, off:off + SH]),
            "ct": ctT,
            "xa": np.ascontiguousarray(xa),
            "xb": np.ascontiguousarray(xb),
            "ohp": np.ascontiguousarray(
                ohr.reshape(NJ, P, C).transpose(1, 0, 2).reshape(P, NJ * C)
            ).astype(BF16_NP),
            "ohs": np.ascontiguousarray(onehot[off:off + SH]),
            "ohsb": np.ascontiguousarray(BIG * onehot[off:off + SH]),
            "pI": pI,
            "cI": cI,
        })
    return in_maps


def finish(targets, per_core_out, per_core_out2):
    t = np.asarray(targets).astype(np.int64)
    cnt = np.bincount(t, minlength=C).astype(np.float64)

    outs = np.stack(per_core_out).astype(np.float64)       # [8, 128, 8]
    l1 = outs[:, :, 0::2].sum()
    pr = outs[:, :, 1::2].sum()

    st = np.stack(per_core_out2).astype(np.float64)        # [8, C, SH]
    st_full = st.transpose(0, 2, 1).reshape(B, C)          # [B, C] = S
    pos_sum = st_full[np.arange(B), t]
    tot_sum = st_full.sum(axis=1)
    pos_cnt = cnt[t]
    pos_mean = pos_sum / pos_cnt
    neg_mean = (tot_sum - pos_sum) / (B - pos_cnt)
    l2 = (pos_mean + np.maximum(IAML_MARGIN - neg_mean, 0.0)).sum()

    loss = np.float32(l1 / B + 0.5 * (l2 / B))
    prec = np.float32(pr / B)
    return (np.asarray(loss, dtype=np.float32), np.asarray(prec, dtype=np.float32))


def kernel(inputs, targets, centers):
    in_maps = make_in_maps(inputs, targets, centers)
    results = _get_runner()(in_maps)
    return finish(targets,
                  [results[i]["out"] for i in range(N_CORES)],
                  [results[i]["out2"] for i in range(N_CORES)])


# revision 26
# speedup vs baseline: 2.1527x; 1.4277x over previous
"""Trainium2 Bass kernel for CompactCenterLoss (B=4096, D=512, C=100, 8 cores).

Math notes (vs the reference):
  dist[i, j] = ||x_i - centers[t_j]|| depends on j only through the class
  t_j, so the first BxB table collapses to a [B, C] table D2:
      dist_ap[i] = D2[i, t_i]                      (all same-class j equal)
      dist_an[i] = min_{c present, c != t_i} D2[i, c]
  Only pdist(x, x) needs the full BxB compute. Its masked row sums are
  obtained per class via a matmul with the one-hot matrix O [B, C]:
      S^T[c, i] = sum_j O[j, c] * dist[j, i]
      pos_sum_i = S^T[t_i, i],  tot_sum_i = sum_c S^T[c, i]

Sharding: batch rows are split across 8 cores (512 rows each). Every core
computes dist^T tiles [128 j x 512 i_shard] against the full (replicated)
input, using a per-core ROTATION of the j axis so the diagonal block is
always device-j-tiles 0..3 -- a single uniform SPMD program for all cores.

The squared-norm terms are folded into the Gram matmul itself as a K=4
rank update (ones rows x hi/lo bf16 split of -0.5*|x|^2), so each dist
tile is PE-matmul -> ACT-sqrt -> PE-matmul with no vector op in the loop.
The diagonal is forced to exact zero (reference's clipped diagonal is
~1e-6, negligible): the diagonal block gets -BIG added before the sqrt
(keeping the sqrt argument positive) and a (1 - I) multiply after it.

Precision: the big Gram matmul runs in bf16 (full PE rate; error on loss2
~1e-4 relative), the small [B, C] table in fp32 (loss1/prec decisions).
"""

import numpy as np
import ml_dtypes
from contextlib import ExitStack

import jax
import concourse.bass as bass
import concourse.tile as tile
import concourse.mybir as mybir
from concourse import bacc
from concourse.bass2jax import install_neuronx_cc_hook, _bass_exec_p, partition_id_tensor

B, D, C = 4096, 512, 100
N_CORES = 8
P = 128
SH = B // N_CORES          # 512 rows per core
NJ = B // P                # 32 j-tiles
KT = D // P                # 4 k-tiles
NCH = SH // P              # 4 i-chunks per core
CCW = 1024                 # xt column-chunk width (for DMA/compute overlap)
NCC = B // CCW             # 4 column chunks
BIG = 1.0e12
IAML_MARGIN = 5.0

f32 = mybir.dt.float32
bf16 = mybir.dt.bfloat16
fp8 = mybir.dt.float8e4
BF16_NP = ml_dtypes.bfloat16
FP8_NP = ml_dtypes.float8_e4m3

Alu = mybir.AluOpType
Act = mybir.ActivationFunctionType


def _build_program():
    nc = bacc.Bacc("TRN2", target_bir_lowering=False, debug=False,
                   enable_asserts=True, num_devices=1)

    # ---- DRAM I/O (per core; host pre-rotates the j axis by the shard offset)
    # X^T in fp8, cols rotated, laid out for DoubleRow: [g, p, s, n] holds
    # element k = g*256 + s*128 + p of column n
    xt8_d = nc.dram_tensor("xt8", [2, P, 2, B], fp8, kind="ExternalInput")
    sqxib_d = nc.dram_tensor("sqxib", [P, SH], f32, kind="ExternalInput")  # |x_i|^2 bcast over partitions
    sqxc_d = nc.dram_tensor("sqxc", [P, NJ], f32, kind="ExternalInput")   # |x_j|^2 per j-tile col (rotated)
    xst_d = nc.dram_tensor("xst", [D, SH], f32, kind="ExternalInput")     # X_shard^T (fp32)
    ct_d = nc.dram_tensor("ct", [D, C], f32, kind="ExternalInput")        # centers^T
    xa_d = nc.dram_tensor("xa", [2, SH], f32, kind="ExternalInput")       # [-0.5|x|^2; 1] i-side
    xb_d = nc.dram_tensor("xb", [2, C], f32, kind="ExternalInput")        # [1; -0.5(|c|^2+BIGabs)]
    ohp_d = nc.dram_tensor("ohp", [P, NJ * C], bf16, kind="ExternalInput")  # one-hot, j-tiled+rotated
    ohs_d = nc.dram_tensor("ohs", [P, NCH * C], f32, kind="ExternalInput")   # one-hot shard, chunk-tiled
    ohsb_d = nc.dram_tensor("ohsb", [P, NCH * C], f32, kind="ExternalInput")  # BIG * same
    pI_d = nc.dram_tensor("pI", [P, P], f32, kind="ExternalInput")        # +2*BIG*I
    cI_d = nc.dram_tensor("cI", [P, P], bf16, kind="ExternalInput")       # 1 - I
    out_d = nc.dram_tensor("out", [P, 2 * NCH], f32, kind="ExternalOutput")
    out2_d = nc.dram_tensor("out2", [C, SH], f32, kind="ExternalOutput")  # S^T

    with tile.TileContext(nc) as tc, ExitStack() as ctx:
        singles = ctx.enter_context(tc.tile_pool(name="singles", bufs=1))
        tmp = ctx.enter_context(tc.tile_pool(name="tmp", bufs=2))
        dpool = ctx.enter_context(tc.tile_pool(name="dist", bufs=4))
        spool = ctx.enter_context(tc.tile_pool(name="sq", bufs=3))
        gram_pool = ctx.enter_context(tc.tile_pool(name="gram", bufs=3, space="PSUM"))
        g1_pool = ctx.enter_context(tc.tile_pool(name="g1", bufs=2, space="PSUM"))
        s_pool = ctx.enter_context(tc.tile_pool(name="sacc", bufs=1, space="PSUM"))

        # ---- DMAs in consumption order: phase B streams first, phase A
        # (which runs last, on a warm PE) loads in the shadow of phase B.
        xt_sb = [[None] * NCC for _ in range(2)]
        for g in range(2):
            t_ = singles.tile([P, 2, CCW], fp8, tag=f"xt{g}_0")
            nc.sync.dma_start(out=t_, in_=xt8_d.ap()[g, :, :, 0:CCW])
            xt_sb[g][0] = t_
        sqxib_sb = singles.tile([P, SH], f32, tag="sqxib")
        nc.sync.dma_start(out=sqxib_sb, in_=sqxib_d.ap())
        sqxc_sb = singles.tile([P, NJ], f32, tag="sqxc")
        nc.sync.dma_start(out=sqxc_sb, in_=sqxc_d.ap())
        pI_sb = singles.tile([P, P], f32, tag="pI")
        nc.sync.dma_start(out=pI_sb, in_=pI_d.ap())
        cI_sb = singles.tile([P, P], bf16, tag="cI")
        nc.sync.dma_start(out=cI_sb, in_=cI_d.ap())
        ohp_sb = singles.tile([P, NJ * C], bf16, tag="ohp")
        nc.sync.dma_start(out=ohp_sb, in_=ohp_d.ap())
        for cc in range(1, NCC):
            for g in range(2):
                t_ = singles.tile([P, 2, CCW], fp8, tag=f"xt{g}_{cc}")
                nc.sync.dma_start(
                    out=t_,
                    in_=xt8_d.ap()[g, :, :, cc * CCW:(cc + 1) * CCW])
                xt_sb[g][cc] = t_
        # phase A inputs (needed only near the end)
        xst_sb = []
        for k in range(KT):
            t_ = singles.tile([P, SH], f32, tag=f"xst{k}")
            nc.sync.dma_start(out=t_, in_=xst_d.ap()[k * P:(k + 1) * P, :])
            xst_sb.append(t_)
        ct_sb = []
        for k in range(KT):
            t_ = singles.tile([P, C], f32, tag=f"ct{k}")
            nc.sync.dma_start(out=t_, in_=ct_d.ap()[k * P:(k + 1) * P, :])
            ct_sb.append(t_)
        xa_sb = singles.tile([2, SH], f32, tag="xa")
        nc.sync.dma_start(out=xa_sb, in_=xa_d.ap())
        xb_sb = singles.tile([2, C], f32, tag="xb")
        nc.sync.dma_start(out=xb_sb, in_=xb_d.ap())
        ohs_sb = singles.tile([P, NCH, C], f32, tag="ohs")
        nc.sync.dma_start(out=ohs_sb, in_=ohs_d.ap().rearrange("p (k c) -> p k c", k=NCH))
        ohsb_sb = singles.tile([P, NCH, C], f32, tag="ohsb")
        nc.sync.dma_start(out=ohsb_sb, in_=ohsb_d.ap().rearrange("p (k c) -> p k c", k=NCH))

        out_sb = singles.tile([P, 2 * NCH], f32, tag="out")

        # ---- phase B: pdist(x, x) tiles + per-class row sums S^T
        # The S matmul for tile t is emitted two tiles late so the in-order
        # PE never waits on the DVE->ACT sqrt chain of its own tile.
        st_acc = s_pool.tile([C, SH], f32, tag="st", name="st_acc")
        dists = [None] * NJ

        def s_matmul(u):
            nc.tensor.matmul(st_acc, ohp_sb[:, u * C:(u + 1) * C], dists[u],
                             start=(u == 0), stop=(u == NJ - 1),
                             skip_group_check=True)

        for t in range(NJ):
            cc, col0 = (t * P) // CCW, (t * P) % CCW
            gram = gram_pool.tile([P, SH], f32)
            for g in range(2):
                nc.tensor.matmul(gram, xt_sb[g][cc][:, :, col0:col0 + P],
                                 xt_sb[g][0][:, :, 0:SH],
                                 start=(g == 0), stop=(g == 1),
                                 perf_mode=mybir.MatmulPerfMode.DoubleRow)
            if t >= 2:
                s_matmul(t - 2)
            # sq = -2*gram + |x_i|^2 ; dist = sqrt(sq + |x_j|^2)
            sq = spool.tile([P, SH], f32)
            nc.vector.scalar_tensor_tensor(out=sq, in0=gram, scalar=-2.0,
                                           in1=sqxib_sb, op0=Alu.mult, op1=Alu.add)
            dist = dpool.tile([P, SH], bf16)
            nc.scalar.activation(out=dist, in_=sq, func=Act.Sqrt,
                                 bias=sqxc_sb[:, t:t + 1], scale=1.0)
            if t < NCH:
                # diagonal block: recompute with +2*BIG on the diagonal
                # (positive sqrt argument), then zero the diagonal exactly
                fx = tmp.tile([P, P], f32, tag="fx")
                nc.vector.tensor_add(fx, sq[:, t * P:(t + 1) * P], pI_sb)
                nc.scalar.activation(out=dist[:, t * P:(t + 1) * P], in_=fx,
                                     func=Act.Sqrt, bias=sqxc_sb[:, t:t + 1],
                                     scale=1.0)
                nc.vector.tensor_tensor(out=dist[:, t * P:(t + 1) * P],
                                        in0=dist[:, t * P:(t + 1) * P],
                                        in1=cI_sb, op=Alu.mult)
            dists[t] = dist
        s_matmul(NJ - 2)

        # ---- phase A (emitted last: PE is warm, and its fp32 matmuls fill
        # the latency of the last dist tile's DVE->ACT chain). All four
        # i-chunks land in one [P, NCH, C] tile so the DVE reductions run
        # once over the whole thing.
        d2a = singles.tile([P, NCH, C], f32, tag="d2a")
        for k in range(NCH):
            g1 = g1_pool.tile([P, C], f32)
            for kt in range(KT):
                nc.tensor.matmul(g1, xst_sb[kt][:, k * P:(k + 1) * P], ct_sb[kt],
                                 start=(kt == 0), stop=False)
            nc.tensor.matmul(g1, xa_sb[:, k * P:(k + 1) * P], xb_sb,
                             start=False, stop=True)
            nc.scalar.activation(out=d2a[:, k, :], in_=g1, func=Act.Sqrt,
                                 bias=0.0, scale=-2.0)
        jk = tmp.tile([P, NCH, C], f32, tag="jk")
        ap = tmp.tile([P, NCH], f32, tag="ap")
        nc.vector.tensor_mul(jk, d2a, ohs_sb)
        nc.vector.tensor_reduce(ap, jk, axis=mybir.AxisListType.X, op=Alu.add)
        jk2 = tmp.tile([P, NCH, C], f32, tag="jk2")
        an = tmp.tile([P, NCH], f32, tag="an")
        nc.vector.tensor_add(jk2, d2a, ohsb_sb)
        nc.vector.tensor_reduce(an, jk2, axis=mybir.AxisListType.X, op=Alu.min)
        diff = tmp.tile([P, NCH], f32, tag="diff")
        nc.vector.tensor_sub(diff, ap, an)
        nc.vector.tensor_scalar_max(out_sb[:, 0:NCH], diff, 0.0)
        nc.vector.tensor_tensor(out=out_sb[:, NCH:2 * NCH],
                                in0=an, in1=ap, op=Alu.is_gt)

        s_matmul(NJ - 1)

        # ---- tail: ship S^T; host finishes loss2
        sts = singles.tile([C, SH], f32, tag="sts")
        nc.vector.tensor_copy(sts, st_acc)
        nc.sync.dma_start(out=out2_d.ap(), in_=sts)
        nc.sync.dma_start(out=out_d.ap(), in_=out_sb)

    nc.compile()
    return nc


_RUNNER = None


def _make_runner():
    """Build the program once and return a cached callable
    in_maps -> list of per-core {"out": ..., "out2": ...}. Mirrors
    concourse.bass2jax.run_bass_via_pjrt but keeps the jitted executable
    alive so repeated kernel() calls don't recompile."""
    from jax.sharding import Mesh, PartitionSpec
    from jax.experimental.shard_map import shard_map

    nc = _build_program()
    install_neuronx_cc_hook()

    partition_name = nc.partition_id_tensor.name if nc.partition_id_tensor else None
    in_names, out_names, out_avals, zero_shapes = [], [], [], []
    for alloc in nc.m.functions[0].allocations:
        if not isinstance(alloc, mybir.MemoryLocationSet):
            continue
        name = alloc.memorylocations[0].name
        if alloc.kind == "ExternalInput":
            if name != partition_name:
                in_names.append(name)
        elif alloc.kind == "ExternalOutput":
            shape = tuple(alloc.tensor_shape)
            dtype = mybir.dt.np(alloc.dtype)
            out_names.append(name)
            out_avals.append(jax.core.ShapedArray(shape, dtype))
            zero_shapes.append((shape, dtype))
    n_params = len(in_names)
    n_outs = len(out_avals)
    all_in_names = list(in_names) + list(out_names)
    if partition_name is not None:
        all_in_names.append(partition_name)
    donate = tuple(range(n_params, n_params + n_outs))

    def _body(*args):
        operands = list(args)
        if partition_name is not None:
            operands.append(partition_id_tensor())
        outs = _bass_exec_p.bind(
            *operands,
            out_avals=tuple(out_avals),
            in_names=tuple(all_in_names),
            out_names=tuple(out_names),
            lowering_input_output_aliases=(),
            sim_require_finite=True,
            sim_require_nnan=True,
            nc=nc,
        )
        return tuple(outs)

    devices = jax.devices()[:N_CORES]
    mesh = Mesh(np.asarray(devices), ("core",))
    in_specs = (PartitionSpec("core"),) * (n_params + n_outs)
    out_specs = (PartitionSpec("core"),) * n_outs
    sharded = jax.jit(
        shard_map(_body, mesh=mesh, in_specs=in_specs, out_specs=out_specs,
                  check_rep=False),
        donate_argnums=donate, keep_unused=True)

    def run(in_maps):
        concat_in = [
            np.concatenate([np.asarray(in_maps[c][name]) for c in range(N_CORES)],
                           axis=0)
            for name in in_names
        ]
        concat_zeros = [np.zeros((N_CORES * s[0], *s[1:]), dt)
                        for (s, dt) in zero_shapes]
        out_arrs = sharded(*concat_in, *concat_zeros)
        return [
            {name: np.asarray(out_arrs[i]).reshape(N_CORES, *out_avals[i].shape)[c]
             for i, name in enumerate(out_names)}
            for c in range(N_CORES)
        ]

    return run


def _get_runner():
    global _RUNNER
    if _RUNNER is None:
        _RUNNER = _make_runner()
    return _RUNNER


def _hilo(v):
    """Split fp32 vector v into bf16 hi/lo with hi+lo ~ v (double-bf16)."""
    hi = v.astype(BF16_NP)
    lo = (v - hi.astype(np.float32)).astype(BF16_NP)
    return hi, lo


def make_in_maps(inputs, targets, centers):
    x = np.ascontiguousarray(np.asarray(inputs, dtype=np.float32))
    t = np.asarray(targets).astype(np.int64)
    c = np.ascontiguousarray(np.asarray(centers, dtype=np.float32))

    sqx = np.sum(x * x, axis=1, dtype=np.float32)          # [B]
    sqc = np.sum(c * c, axis=1, dtype=np.float32)          # [C]
    cnt = np.bincount(t, minlength=C).astype(np.float32)   # [C]
    absent = (cnt == 0).astype(np.float32)
    onehot = (t[:, None] == np.arange(C)[None, :]).astype(np.float32)  # [B, C]

    xtT = np.ascontiguousarray(x.T)                        # [D, B]
    ctT = np.ascontiguousarray(c.T)                        # [D, C]
    pI = (2.0 * BIG) * np.eye(P, dtype=np.float32)
    cI = (1.0 - np.eye(P)).astype(BF16_NP)

    in_maps = []
    for core in range(N_CORES):
        off = core * SH
        xr_cols = np.roll(xtT, -off, axis=1)
        ohr = np.roll(onehot, -off, axis=0)
        sqxr = np.roll(sqx, -off)
        xt8 = np.ascontiguousarray(
            xr_cols.astype(FP8_NP).reshape(2, 2, P, B).transpose(0, 2, 1, 3))
        xa = np.stack([-0.5 * sqx[off:off + SH],
                       np.ones(SH, np.float32)])              # [2, SH]
        xb = np.stack([np.ones(C, np.float32),
                       -0.5 * (sqc + BIG * absent)])          # [2, C]
        ohs_t = np.ascontiguousarray(
            onehot[off:off + SH].reshape(NCH, P, C).transpose(1, 0, 2)
            .reshape(P, NCH * C))
        in_maps.append({
            "xt8": xt8,
            "sqxib": np.tile(sqx[off:off + SH][None, :], (P, 1)),
            "sqxc": np.ascontiguousarray(sqxr.reshape(NJ, P).T),
            "xst": np.ascontiguousarray(xtT[:, off:off + SH]),
            "ct": ctT,
            "xa": np.ascontiguousarray(xa),
            "xb": np.ascontiguousarray(xb),
            "ohp": np.ascontiguousarray(
                ohr.reshape(NJ, P, C).transpose(1, 0, 2).reshape(P, NJ * C)
            ).astype(BF16_NP),
            "ohs": ohs_t,
            "ohsb": np.ascontiguousarray(BIG * ohs_t),
            "pI": pI,
            "cI": cI,
        })
    return in_maps


def finish(targets, per_core_out, per_core_out2):
    t = np.asarray(targets).astype(np.int64)
    cnt = np.bincount(t, minlength=C).astype(np.float64)

    outs = np.stack(per_core_out).astype(np.float64)       # [8, 128, 8]
    l1 = outs[:, :, 0:NCH].sum()
    pr = outs[:, :, NCH:2 * NCH].sum()

    st = np.stack(per_core_out2).astype(np.float64)        # [8, C, SH]
    st_full = st.transpose(0, 2, 1).reshape(B, C)          # [B, C] = S
    pos_sum = st_full[np.arange(B), t]
    tot_sum = st_full.sum(axis=1)
    pos_cnt = cnt[t]
    pos_mean = pos_sum / pos_cnt
    neg_mean = (tot_sum - pos_sum) / (B - pos_cnt)
    l2 = (pos_mean + np.maximum(IAML_MARGIN - neg_mean, 0.0)).sum()

    loss = np.float32(l1 / B + 0.5 * (l2 / B))
    prec = np.float32(pr / B)
    return (np.asarray(loss, dtype=np.float32), np.asarray(prec, dtype=np.float32))


def kernel(inputs, targets, centers):
    in_maps = make_in_maps(inputs, targets, centers)
    results = _get_runner()(in_maps)
    return finish(targets,
                  [results[i]["out"] for i in range(N_CORES)],
                  [results[i]["out2"] for i in range(N_CORES)])


# revision 37
# speedup vs baseline: 2.2379x; 1.0396x over previous
"""Trainium2 Bass kernel for CompactCenterLoss (B=4096, D=512, C=100, 8 cores).

Math notes (vs the reference):
  dist[i, j] = ||x_i - centers[t_j]|| depends on j only through the class
  t_j, so the first BxB table collapses to a [B, C] table D2:
      dist_ap[i] = D2[i, t_i]                      (all same-class j equal)
      dist_an[i] = min_{c present, c != t_i} D2[i, c]
  Only pdist(x, x) needs the full BxB compute. Its masked row sums are
  obtained per class via a matmul with the one-hot matrix O [B, C]:
      S^T[c, i] = sum_j O[j, c] * dist[j, i]
      pos_sum_i = S^T[t_i, i],  tot_sum_i = sum_c S^T[c, i]

Sharding: batch rows are split across 8 cores (512 rows each). Every core
computes dist^T tiles [128 j x 512 i_shard] against the full (replicated)
input, using a per-core ROTATION of the j axis so the diagonal block is
always device-j-tiles 0..3 -- a single uniform SPMD program for all cores.

The squared-norm terms are folded into the Gram matmul itself as a K=4
rank update (ones rows x hi/lo bf16 split of -0.5*|x|^2), so each dist
tile is PE-matmul -> ACT-sqrt -> PE-matmul with no vector op in the loop.
The diagonal is forced to exact zero (reference's clipped diagonal is
~1e-6, negligible): the diagonal block gets -BIG added before the sqrt
(keeping the sqrt argument positive) and a (1 - I) multiply after it.

Precision: the big Gram matmul runs in bf16 (full PE rate; error on loss2
~1e-4 relative), the small [B, C] table in fp32 (loss1/prec decisions).
"""

import numpy as np
import ml_dtypes
from contextlib import ExitStack

import jax
import concourse.bass as bass
import concourse.tile as tile
import concourse.mybir as mybir
from concourse import bacc
from concourse.bass2jax import install_neuronx_cc_hook, _bass_exec_p, partition_id_tensor

B, D, C = 4096, 512, 100
N_CORES = 8
P = 128
SH = B // N_CORES          # 512 rows per core
NJ = B // P                # 32 j-tiles
KT = D // P                # 4 k-tiles
NCH = SH // P              # 4 i-chunks per core
CCW = 1024                 # xt column-chunk width (for DMA/compute overlap)
NCC = B // CCW             # 4 column chunks
BIG = 1.0e12
DIAG_SQ = 1.0e4   # sqrt argument forced onto the (zeroed) diagonal: large vs
                  # Gram noise, and sqrt(1e4)=100 stays finite in fp8 (max 240)
IAML_MARGIN = 5.0

f32 = mybir.dt.float32
bf16 = mybir.dt.bfloat16
fp8 = mybir.dt.float8e4
BF16_NP = ml_dtypes.bfloat16
FP8_NP = ml_dtypes.float8_e4m3

Alu = mybir.AluOpType
Act = mybir.ActivationFunctionType


def _build_program():
    nc = bacc.Bacc("TRN2", target_bir_lowering=False, debug=False,
                   enable_asserts=True, num_devices=1)

    # ---- DRAM I/O (per core; host pre-rotates the j axis by the shard offset)
    # X^T in fp8, cols rotated, laid out for DoubleRow: [g, p, s, n] holds
    # element k = g*256 + s*128 + p of column n
    xt8_d = nc.dram_tensor("xt8", [2, P, 2, B], fp8, kind="ExternalInput")
    xe_d = nc.dram_tensor("xe", [4, B], bf16, kind="ExternalInput")       # [hi;lo;1;1] j-side
    xr_d = nc.dram_tensor("xr", [4, SH], bf16, kind="ExternalInput")      # [1;1;hi;lo] i-side
    xst_d = nc.dram_tensor("xst", [D, SH], f32, kind="ExternalInput")     # X_shard^T (fp32)
    ct_d = nc.dram_tensor("ct", [D, C], f32, kind="ExternalInput")        # centers^T
    xa_d = nc.dram_tensor("xa", [2, SH], f32, kind="ExternalInput")       # [-0.5|x|^2; 1] i-side
    xb_d = nc.dram_tensor("xb", [2, C], f32, kind="ExternalInput")        # [1; -0.5(|c|^2+BIGabs)]
    ohp_d = nc.dram_tensor("ohp", [P, NJ * C], fp8, kind="ExternalInput")  # one-hot, j-tiled+rotated
    ohs_d = nc.dram_tensor("ohs", [P, NCH * C], f32, kind="ExternalInput")   # one-hot shard, chunk-tiled
    ohsb_d = nc.dram_tensor("ohsb", [P, NCH * C], f32, kind="ExternalInput")  # BIG * same
    nI_d = nc.dram_tensor("nI", [P, P], f32, kind="ExternalInput")        # -0.5*DIAG_SQ*I
    cI_d = nc.dram_tensor("cI", [P, P], fp8, kind="ExternalInput")        # 1 - I
    out_d = nc.dram_tensor("out", [P, 2 * NCH], f32, kind="ExternalOutput")
    out2_d = nc.dram_tensor("out2", [C, SH], f32, kind="ExternalOutput")  # S^T

    with tile.TileContext(nc) as tc, ExitStack() as ctx:
        singles = ctx.enter_context(tc.tile_pool(name="singles", bufs=1))
        tmp = ctx.enter_context(tc.tile_pool(name="tmp", bufs=2))
        dpool = ctx.enter_context(tc.tile_pool(name="dist", bufs=4))
        gram_pool = ctx.enter_context(tc.tile_pool(name="gram", bufs=2, space="PSUM"))
        g1_pool = ctx.enter_context(tc.tile_pool(name="g1", bufs=2, space="PSUM"))
        s_pool = ctx.enter_context(tc.tile_pool(name="sacc", bufs=1, space="PSUM"))

        # ---- DMAs in consumption order: phase B streams first, phase A
        # (which runs last, on a warm PE) loads in the shadow of phase B.
        xt_sb = [[None] * NCC for _ in range(2)]
        for g in range(2):
            t_ = singles.tile([P, 2, CCW], fp8, tag=f"xt{g}_0")
            nc.sync.dma_start(out=t_, in_=xt8_d.ap()[g, :, :, 0:CCW])
            xt_sb[g][0] = t_
        xe_sb = singles.tile([4, B], bf16, tag="xe")
        nc.sync.dma_start(out=xe_sb, in_=xe_d.ap())
        xr_sb = singles.tile([4, SH], bf16, tag="xr")
        nc.sync.dma_start(out=xr_sb, in_=xr_d.ap())
        nI_sb = singles.tile([P, P], f32, tag="nI")
        nc.sync.dma_start(out=nI_sb, in_=nI_d.ap())
        cI_sb = singles.tile([P, P], fp8, tag="cI")
        nc.sync.dma_start(out=cI_sb, in_=cI_d.ap())
        # padded to a 16-byte middle-dim stride (dual-fp8 LDWEIGHTS rule)
        CPAD = 112
        ohp_sb = singles.tile([P, NJ, CPAD], fp8, tag="ohp")
        nc.sync.dma_start(out=ohp_sb[:, :, 0:C],
                          in_=ohp_d.ap().rearrange("p (t c) -> p t c", t=NJ))
        for cc in range(1, NCC):
            for g in range(2):
                t_ = singles.tile([P, 2, CCW], fp8, tag=f"xt{g}_{cc}")
                nc.sync.dma_start(
                    out=t_,
                    in_=xt8_d.ap()[g, :, :, cc * CCW:(cc + 1) * CCW])
                xt_sb[g][cc] = t_
        # phase A inputs (needed only near the end)
        xst_sb = []
        for k in range(KT):
            t_ = singles.tile([P, SH], f32, tag=f"xst{k}")
            nc.sync.dma_start(out=t_, in_=xst_d.ap()[k * P:(k + 1) * P, :])
            xst_sb.append(t_)
        ct_sb = []
        for k in range(KT):
            t_ = singles.tile([P, C], f32, tag=f"ct{k}")
            nc.sync.dma_start(out=t_, in_=ct_d.ap()[k * P:(k + 1) * P, :])
            ct_sb.append(t_)
        xa_sb = singles.tile([2, SH], f32, tag="xa")
        nc.sync.dma_start(out=xa_sb, in_=xa_d.ap())
        xb_sb = singles.tile([2, C], f32, tag="xb")
        nc.sync.dma_start(out=xb_sb, in_=xb_d.ap())
        ohs_sb = singles.tile([P, NCH, C], f32, tag="ohs")
        nc.sync.dma_start(out=ohs_sb, in_=ohs_d.ap().rearrange("p (k c) -> p k c", k=NCH))
        ohsb_sb = singles.tile([P, NCH, C], f32, tag="ohsb")
        nc.sync.dma_start(out=ohsb_sb, in_=ohsb_d.ap().rearrange("p (k c) -> p k c", k=NCH))

        out_sb = singles.tile([P, 2 * NCH], f32, tag="out")

        # ---- phase B: pdist(x, x) tiles + per-class row sums S^T
        # Tiles are processed in PAIRS sharing one 2-bank PSUM tile, one ACT
        # sqrt and one DoubleRow S matmul. The squared-norm terms are folded
        # into the Gram accumulation as a bf16 K=4 rank update so the sqrt is
        # bias-free (pairable) and the DVE stays out of the inner loop. The S
        # matmul for pair u is emitted two pairs late so the in-order PE
        # never waits on its own pair's sqrt.
        NP_ = NJ // 2
        st_acc = s_pool.tile([C, SH], f32, tag="st", name="st_acc")
        dist2s = [None] * NP_

        def s_matmul(u):
            nc.tensor.matmul(st_acc, ohp_sb[:, 2 * u:2 * u + 2, 0:C],
                             dist2s[u],
                             start=(u == 0), stop=(u == NP_ - 1),
                             perf_mode=mybir.MatmulPerfMode.DoubleRow,
                             skip_group_check=True)

        for pr in range(NP_):
            gram2 = gram_pool.tile([P, 2, SH], f32)
            for s in range(2):
                t = 2 * pr + s
                cc, col0 = (t * P) // CCW, (t * P) % CCW
                for g in range(2):
                    nc.tensor.matmul(gram2[:, s, :],
                                     xt_sb[g][cc][:, :, col0:col0 + P],
                                     xt_sb[g][0][:, :, 0:SH],
                                     start=(g == 0), stop=False,
                                     perf_mode=mybir.MatmulPerfMode.DoubleRow)
                # bf16 K=4 rank update folds in -0.5(|x_j|^2 + |x_i|^2)
                nc.tensor.matmul(gram2[:, s, :], xe_sb[:, t * P:(t + 1) * P],
                                 xr_sb, start=False, stop=True)
            if pr >= 2:
                s_matmul(pr - 2)
            dist2 = dpool.tile([P, 2, SH], fp8)
            nc.scalar.activation(out=dist2, in_=gram2, func=Act.Sqrt,
                                 bias=0.0, scale=-2.0)
            if pr < 2:
                # diagonal blocks (tiles 0..3): recompute with the sqrt
                # argument pushed to +DIAG_SQ, then zero the diagonal exactly
                for s in range(2):
                    t = 2 * pr + s
                    fx = tmp.tile([P, P], f32, tag="fx")
                    nc.vector.tensor_add(fx, gram2[:, s, t * P:(t + 1) * P],
                                         nI_sb)
                    nc.scalar.activation(out=dist2[:, s, t * P:(t + 1) * P],
                                         in_=fx, func=Act.Sqrt,
                                         bias=0.0, scale=-2.0)
                    nc.vector.tensor_tensor(
                        out=dist2[:, s, t * P:(t + 1) * P],
                        in0=dist2[:, s, t * P:(t + 1) * P],
                        in1=cI_sb, op=Alu.mult)
            dist2s[pr] = dist2
        s_matmul(NP_ - 2)

        # ---- phase A (emitted last: PE is warm, and its fp32 matmuls fill
        # the latency of the last dist tile's DVE->ACT chain). All four
        # i-chunks land in one [P, NCH, C] tile so the DVE reductions run
        # once over the whole thing.
        d2a = singles.tile([P, NCH, C], f32, tag="d2a")
        for k in range(NCH):
            g1 = g1_pool.tile([P, C], f32)
            for kt in range(KT):
                nc.tensor.matmul(g1, xst_sb[kt][:, k * P:(k + 1) * P], ct_sb[kt],
                                 start=(kt == 0), stop=False)
            nc.tensor.matmul(g1, xa_sb[:, k * P:(k + 1) * P], xb_sb,
                             start=False, stop=True)
            nc.scalar.activation(out=d2a[:, k, :], in_=g1, func=Act.Sqrt,
                                 bias=0.0, scale=-2.0)
        jk = tmp.tile([P, NCH, C], f32, tag="jk")
        ap = tmp.tile([P, NCH], f32, tag="ap")
        nc.vector.tensor_mul(jk, d2a, ohs_sb)
        nc.vector.tensor_reduce(ap, jk, axis=mybir.AxisListType.X, op=Alu.add)
        jk2 = tmp.tile([P, NCH, C], f32, tag="jk2")
        an = tmp.tile([P, NCH], f32, tag="an")
        nc.vector.tensor_add(jk2, d2a, ohsb_sb)
        nc.vector.tensor_reduce(an, jk2, axis=mybir.AxisListType.X, op=Alu.min)
        diff = tmp.tile([P, NCH], f32, tag="diff")
        nc.vector.tensor_sub(diff, ap, an)
        nc.vector.tensor_scalar_max(out_sb[:, 0:NCH], diff, 0.0)
        nc.vector.tensor_tensor(out=out_sb[:, NCH:2 * NCH],
                                in0=an, in1=ap, op=Alu.is_gt)

        s_matmul(NP_ - 1)

        # ---- tail: ship S^T; host finishes loss2
        sts = singles.tile([C, SH], f32, tag="sts")
        nc.vector.tensor_copy(sts, st_acc)
        nc.sync.dma_start(out=out2_d.ap(), in_=sts)
        nc.sync.dma_start(out=out_d.ap(), in_=out_sb)

    nc.compile()
    return nc


_RUNNER = None


def _make_runner():
    """Build the program once and return a cached callable
    in_maps -> list of per-core {"out": ..., "out2": ...}. Mirrors
    concourse.bass2jax.run_bass_via_pjrt but keeps the jitted executable
    alive so repeated kernel() calls don't recompile."""
    from jax.sharding import Mesh, PartitionSpec
    from jax.experimental.shard_map import shard_map

    nc = _build_program()
    install_neuronx_cc_hook()

    partition_name = nc.partition_id_tensor.name if nc.partition_id_tensor else None
    in_names, out_names, out_avals, zero_shapes = [], [], [], []
    for alloc in nc.m.functions[0].allocations:
        if not isinstance(alloc, mybir.MemoryLocationSet):
            continue
        name = alloc.memorylocations[0].name
        if alloc.kind == "ExternalInput":
            if name != partition_name:
                in_names.append(name)
        elif alloc.kind == "ExternalOutput":
            shape = tuple(alloc.tensor_shape)
            dtype = mybir.dt.np(alloc.dtype)
            out_names.append(name)
            out_avals.append(jax.core.ShapedArray(shape, dtype))
            zero_shapes.append((shape, dtype))
    n_params = len(in_names)
    n_outs = len(out_avals)
    all_in_names = list(in_names) + list(out_names)
    if partition_name is not None:
        all_in_names.append(partition_name)
    donate = tuple(range(n_params, n_params + n_outs))

    def _body(*args):
        operands = list(args)
        if partition_name is not None:
            operands.append(partition_id_tensor())
        outs = _bass_exec_p.bind(
            *operands,
            out_avals=tuple(out_avals),
            in_names=tuple(all_in_names),
            out_names=tuple(out_names),
            lowering_input_output_aliases=(),
            sim_require_finite=True,
            sim_require_nnan=True,
            nc=nc,
        )
        return tuple(outs)

    devices = jax.devices()[:N_CORES]
    mesh = Mesh(np.asarray(devices), ("core",))
    in_specs = (PartitionSpec("core"),) * (n_params + n_outs)
    out_specs = (PartitionSpec("core"),) * n_outs
    sharded = jax.jit(
        shard_map(_body, mesh=mesh, in_specs=in_specs, out_specs=out_specs,
                  check_rep=False),
        donate_argnums=donate, keep_unused=True)

    def run(in_maps):
        concat_in = [
            np.concatenate([np.asarray(in_maps[c][name]) for c in range(N_CORES)],
                           axis=0)
            for name in in_names
        ]
        concat_zeros = [np.zeros((N_CORES * s[0], *s[1:]), dt)
                        for (s, dt) in zero_shapes]
        out_arrs = sharded(*concat_in, *concat_zeros)
        return [
            {name: np.asarray(out_arrs[i]).reshape(N_CORES, *out_avals[i].shape)[c]
             for i, name in enumerate(out_names)}
            for c in range(N_CORES)
        ]

    return run


def _get_runner():
    global _RUNNER
    if _RUNNER is None:
        _RUNNER = _make_runner()
    return _RUNNER


def _hilo(v):
    """Split fp32 vector v into bf16 hi/lo with hi+lo ~ v (double-bf16)."""
    hi = v.astype(BF16_NP)
    lo = (v - hi.astype(np.float32)).astype(BF16_NP)
    return hi, lo


def make_in_maps(inputs, targets, centers):
    x = np.ascontiguousarray(np.asarray(inputs, dtype=np.float32))
    t = np.asarray(targets).astype(np.int64)
    c = np.ascontiguousarray(np.asarray(centers, dtype=np.float32))

    sqx = np.sum(x * x, axis=1, dtype=np.float32)          # [B]
    sqc = np.sum(c * c, axis=1, dtype=np.float32)          # [C]
    cnt = np.bincount(t, minlength=C).astype(np.float32)   # [C]
    absent = (cnt == 0).astype(np.float32)
    onehot = (t[:, None] == np.arange(C)[None, :]).astype(np.float32)  # [B, C]

    xtT = np.ascontiguousarray(x.T)                        # [D, B]
    ctT = np.ascontiguousarray(c.T)                        # [D, C]
    nI = (-0.5 * DIAG_SQ) * np.eye(P, dtype=np.float32)
    cI = (1.0 - np.eye(P)).astype(FP8_NP)
    ones_bf = np.ones(B, dtype=BF16_NP)

    in_maps = []
    for core in range(N_CORES):
        off = core * SH
        xr_cols = np.roll(xtT, -off, axis=1)
        ohr = np.roll(onehot, -off, axis=0)
        sqxr = np.roll(sqx, -off)
        xt8 = np.ascontiguousarray(
            xr_cols.astype(FP8_NP).reshape(2, 2, P, B).transpose(0, 2, 1, 3))
        hi_j, lo_j = _hilo(-0.5 * sqxr)
        hi_i, lo_i = _hilo(-0.5 * sqx[off:off + SH])
        xe = np.stack([hi_j, lo_j, ones_bf, ones_bf])         # [4, B]
        xr_ = np.stack([np.ones(SH, BF16_NP), np.ones(SH, BF16_NP),
                        hi_i, lo_i])                          # [4, SH]
        xa = np.stack([-0.5 * sqx[off:off + SH],
                       np.ones(SH, np.float32)])              # [2, SH]
        xb = np.stack([np.ones(C, np.float32),
                       -0.5 * (sqc + BIG * absent)])          # [2, C]
        ohs_t = np.ascontiguousarray(
            onehot[off:off + SH].reshape(NCH, P, C).transpose(1, 0, 2)
            .reshape(P, NCH * C))
        in_maps.append({
            "xt8": xt8,
            "xe": np.ascontiguousarray(xe),
            "xr": np.ascontiguousarray(xr_),
            "xst": np.ascontiguousarray(xtT[:, off:off + SH]),
            "ct": ctT,
            "xa": np.ascontiguousarray(xa),
            "xb": np.ascontiguousarray(xb),
            "ohp": np.ascontiguousarray(
                ohr.reshape(NJ, P, C).transpose(1, 0, 2).reshape(P, NJ * C)
            ).astype(FP8_NP),
            "ohs": ohs_t,
            "ohsb": np.ascontiguousarray(BIG * ohs_t),
            "nI": nI,
            "cI": cI,
        })
    return in_maps


def finish(targets, per_core_out, per_core_out2):
    t = np.asarray(targets).astype(np.int64)
    cnt = np.bincount(t, minlength=C).astype(np.float64)

    outs = np.stack(per_core_out).astype(np.float64)       # [8, 128, 8]
    l1 = outs[:, :, 0:NCH].sum()
    pr = outs[:, :, NCH:2 * NCH].sum()

    st = np.stack(per_core_out2).astype(np.float64)        # [8, C, SH]
    st_full = st.transpose(0, 2, 1).reshape(B, C)          # [B, C] = S
    pos_sum = st_full[np.arange(B), t]
    tot_sum = st_full.sum(axis=1)
    pos_cnt = cnt[t]
    pos_mean = pos_sum / pos_cnt
    neg_mean = (tot_sum - pos_sum) / (B - pos_cnt)
    l2 = (pos_mean + np.maximum(IAML_MARGIN - neg_mean, 0.0)).sum()

    loss = np.float32(l1 / B + 0.5 * (l2 / B))
    prec = np.float32(pr / B)
    return (np.asarray(loss, dtype=np.float32), np.asarray(prec, dtype=np.float32))


def kernel(inputs, targets, centers):
    in_maps = make_in_maps(inputs, targets, centers)
    results = _get_runner()(in_maps)
    return finish(targets,
                  [results[i]["out"] for i in range(N_CORES)],
                  [results[i]["out2"] for i in range(N_CORES)])
